# revision 23
# baseline (speedup 1.0000x reference)
"""Bass/Tile kernel for a 4-layer decoder transformer + 32k-vocab LM head on 8 trn2 cores.

Sharding: data-parallel over batch in pairs (core c -> batch c//2), with the
lm_head vocab dim split across each pair (core c -> vocab half c%2). Tiny
transformer params replicated; each core computes the 64-dim transformer for
its sequence and produces [1024, 16000] bf16 logits. Host reassembles
[4, 1024, 32000] f32.

v2 redesign (vs v1 310us): the kernel is PSUM-evacuation bound (every f32
PSUM result needs a DVE/ACT op whose cost = free-size cols + fixed ~0.3us).
  - LM head: one 1000-col bf16 matmul per vocab chunk into per-engine 2-bank
    PSUM tiles (lm_dve/lm_act) so DVE and ACT evacuate concurrently without
    shared-tile read serialization; evac ratio is tunable.
  - Scores: [128,2,512] f32 PSUM tile; ONE Exp per bank-pair (1024 cols)
    halves ACT fixed costs. Causal mask still accumulated via identity mm.
  - av recomputed as out[t, hd] (rhs = v-slice, 32-col outs) -> 4x cheaper on
    PE than [hd, t]; normalization via strided reciprocal + ONE 0-stride
    broadcast tensor_tensor; transpose back for the wo projection.
  - All small evacs batched over chunk pairs/quads (kv fused k+v, batched
    transpose evacuation, batched residual adds, batched relu) to amortize
    per-op fixed costs.
  - b1 folded into w1 row 64 (ones row of xn2t) - relu needs no bias.
  - LN rstd (0x5f3759df + 2 Newton steps) batched over 4 chunks.
"""

import os

import numpy as np
import ml_dtypes

import concourse.bass as bass
import concourse.mybir as mybir
import concourse.tile as tile
from concourse import bacc
from concourse.bass import ts
from concourse.bass_utils import run_bass_kernel_spmd

# model dims
T = 1024
C = 64
H = 4
D = 16
L = 4
FF = 256
V = 32000
VH = V // 2          # per-core vocab half
NT = T // 128        # 8 token chunks
VC = 1000            # lm-head vocab chunk per matmul (one 2-bank psum tile)
NVC = VH // VC       # 16 vocab chunks per core
SG = 4000            # logits staged per output DMA
SCALE = 1.0 / (C ** 0.5)
NEG = -1.0e9

F32 = mybir.dt.float32
BF16 = mybir.dt.bfloat16
I32 = mybir.dt.int32

# of the 16 VC-chunks per token chunk, how many are evacuated by DVE
# (the rest go to ACT) -- balance knob
LM_DVE = 8

_CACHE = {}
LAST_RESULTS = None


def _build(apply_bo, apply_b2):
    nc = bacc.Bacc("TRN2", target_bir_lowering=False, debug=False)

    # ---- DRAM I/O ----
    idx_d = nc.dram_tensor("idx", [128, NT], I32, kind="ExternalInput").ap()
    temb_d = nc.dram_tensor("tok_emb", [V, C], F32, kind="ExternalInput").ap()
    pemb_d = nc.dram_tensor("pos_emb", [T, C], F32, kind="ExternalInput").ap()
    wq_d = nc.dram_tensor("wq", [65, L, H, 128], BF16, kind="ExternalInput").ap()
    wk_d = nc.dram_tensor("wk", [65, L, 128], BF16, kind="ExternalInput").ap()
    wv_d = nc.dram_tensor("wv", [65, L, 128], BF16, kind="ExternalInput").ap()
    wo_d = nc.dram_tensor("wo", [64, L, C], BF16, kind="ExternalInput").ap()
    w1_d = nc.dram_tensor("w1", [65, L, FF], BF16, kind="ExternalInput").ap()
    w2_d = nc.dram_tensor("w2", [128, L, 2, C], BF16, kind="ExternalInput").ap()
    wlm_d = nc.dram_tensor("wlm", [65, VH], BF16, kind="ExternalInput").ap()
    mask_d = nc.dram_tensor("maskt", [128, 512], BF16, kind="ExternalInput").ap()
    idb_d = nc.dram_tensor("identb", [128, 128], BF16, kind="ExternalInput").ap()
    bo_d = b2_d = None
    if apply_bo:
        bo_d = nc.dram_tensor("bo_bc", [L, C], F32, kind="ExternalInput").ap()
    if apply_b2:
        b2_d = nc.dram_tensor("b2_bc", [L, C], F32, kind="ExternalInput").ap()
    out_d = nc.dram_tensor("logits", [T, VH], BF16, kind="ExternalOutput").ap()

    with tile.TileContext(nc) as tc:
        with (
            tc.tile_pool(name="singles", bufs=1) as singles,
            tc.tile_pool(name="sb2", bufs=3) as sb2,
            tc.tile_pool(name="sb4", bufs=4) as sb4,
            tc.tile_pool(name="ptp", bufs=5) as ptp,
            tc.tile_pool(name="stage", bufs=4) as stagep,
            tc.tile_pool(name="pss", bufs=1, space="PSUM") as pss,
        ):
            # ---- persistent PSUM tiles, manually slotted (region-granular
            # deps avoid pool-ring serialization) ----
            # SC1/SC2: alternating 2-bank pair-tiles for q / score-pairs /
            # mlp-h / at2 -- exp of tile A overlaps matmul fill of tile B
            SC1 = pss.tile([128, 2, 512], F32, name="SC1")     # banks 0-1
            SC2 = pss.tile([128, 2, 512], F32, name="SC2")     # banks 2-3
            LMD = pss.tile([128, 512], F32, name="LMD")        # bank 4
            LMA = pss.tile([128, 512], F32, name="LMA")        # bank 5
            # TRS: bf16 transpose staging + xo/y f32 region via bitcast
            TRS = pss.tile([128, 1024], BF16, name="TRS")      # bank 6
            SMALL = pss.tile([128, 512], F32, name="SMALL")    # bank 7
            xoy = TRS[:, 768:1024].bitcast(F32)                # [128,128] f32
            _scu = [0]

            def pair_use():
                t = SC1 if (_scu[0] & 1) == 0 else SC2
                _scu[0] += 1
                return t
            # ---- resident SBUF tensors ----
            wq_sb = singles.tile([65, L, H, 128], BF16, name="wq_sb")
            wk_sb = singles.tile([65, L, 128], BF16, name="wk_sb")
            wv_sb = singles.tile([65, L, 128], BF16, name="wv_sb")
            wo_sb = singles.tile([64, L, C], BF16, name="wo_sb")
            w1_sb = singles.tile([65, L, FF], BF16, name="w1_sb")
            w2_sb = singles.tile([128, L, 2, C], BF16, name="w2_sb")
            wlm_sb = singles.tile([65, VH], BF16, name="wlm_sb")
            mask_sb = singles.tile([128, 512], BF16, name="mask_sb")
            idb_sb = singles.tile([128, 128], BF16, name="idb_sb")
            idx_sb = singles.tile([128, NT], I32, name="idx_sb")
            x_sb = singles.tile([128, NT, C], F32, name="x_sb")
            # fused k|v cache: [0:128] k as [dstack, t]; [128:256] v as [s, dstack]
            kv_sb = [singles.tile([128, NT, 256], BF16, name=f"kv{l}") for l in range(L)]
            # transposed-normalized-x slots with persistent ones row (row 64)
            xnt_sl = singles.tile([65, 4, 128], BF16, name="xnt_sl")
            xn2t_sl = singles.tile([65, 4, 128], BF16, name="xn2t_sl")
            att_sl = singles.tile([64, 4, 128], BF16, name="att_sl")
            xt_sl = singles.tile([65, 4, 128], BF16, name="xt_sl")

            nc.sync.dma_start(out=idx_sb, in_=idx_d)
            pos_sb = singles.tile([128, NT, C], F32, name="pos_sb")
            nc.sync.dma_start(
                out=pos_sb,
                in_=bass.AP(tensor=pemb_d.tensor, offset=pemb_d.offset,
                            ap=[[C, 128], [128 * C, NT], [1, C]]))
            nc.sync.dma_start(out=wq_sb, in_=wq_d)
            nc.sync.dma_start(out=wk_sb, in_=wk_d)
            nc.sync.dma_start(out=wv_sb, in_=wv_d)
            nc.sync.dma_start(out=idb_sb, in_=idb_d)
            nc.sync.dma_start(out=mask_sb, in_=mask_d)
            nc.sync.dma_start(out=wo_sb, in_=wo_d)
            nc.sync.dma_start(out=w1_sb, in_=w1_d)
            nc.sync.dma_start(out=w2_sb, in_=w2_d)
            nc.sync.dma_start(out=wlm_sb, in_=wlm_d)
            nc.vector.memset(xnt_sl[64:65, :, :], 1.0)
            nc.vector.memset(xn2t_sl[64:65, :, :], 1.0)
            nc.vector.memset(xt_sl[64:65, :, :], 1.0)

            bo_sb = b2_sb = None
            if apply_bo:
                bo_sb = singles.tile([128, L, C], F32, name="bo_sb")
                for l in range(L):
                    row = bo_d[l: l + 1, :]
                    nc.gpsimd.dma_start(
                        out=bo_sb[:, l, :],
                        in_=bass.AP(tensor=row.tensor, offset=row.offset,
                                    ap=[[0, 128], [1, C]]))
            if apply_b2:
                b2_sb = singles.tile([128, L, C], F32, name="b2_sb")
                for l in range(L):
                    row = b2_d[l: l + 1, :]
                    nc.gpsimd.dma_start(
                        out=b2_sb[:, l, :],
                        in_=bass.AP(tensor=row.tensor, offset=row.offset,
                                    ap=[[0, 128], [1, C]]))

            # ---- embeddings: x = tok_emb[idx] + pos_emb ----
            for i in range(NT):
                nc.gpsimd.indirect_dma_start(
                    out=x_sb[:, i, :],
                    out_offset=None,
                    in_=temb_d,
                    in_offset=bass.IndirectOffsetOnAxis(ap=idx_sb[:, i:i + 1], axis=0),
                )
                nc.vector.tensor_add(out=x_sb[:, i, :], in0=x_sb[:, i, :],
                                     in1=pos_sb[:, i, :])

            def batched_rstd(mv_g, n, name):
                """rstd[:, 0:n] = (var + eps)^-0.5 for n chunks at once.
                0x5f3759df bit-seed + two Newton steps, all on DVE."""
                veps = sb2.tile([128, n], F32, name=f"veps{name}")
                sh = sb2.tile([128, n], I32, name=f"sh{name}")
                rt = sb2.tile([128, n], F32, name=f"rt{name}")
                rstd = sb2.tile([128, n], F32, name=f"rstd{name}")
                var_ap = bass.AP(tensor=mv_g.tensor, offset=mv_g.offset + 1,
                                 ap=[mv_g.ap[0], [2, n]])
                nc.vector.tensor_scalar(out=veps, in0=var_ap,
                                        scalar1=1e-5, scalar2=None,
                                        op0=mybir.AluOpType.add)
                nc.vector.tensor_scalar(out=sh, in0=veps.bitcast(I32),
                                        scalar1=1, scalar2=None,
                                        op0=mybir.AluOpType.arith_shift_right)
                nc.vector.tensor_scalar(out=sh, in0=sh, scalar1=0, scalar2=None,
                                        op0=mybir.AluOpType.bitwise_not)
                nc.vector.tensor_scalar(out=rstd.bitcast(I32), in0=sh,
                                        scalar1=0x5f3759df + 1, scalar2=None,
                                        op0=mybir.AluOpType.add)
                for _ in range(2):
                    nc.vector.tensor_tensor(out=rt, in0=rstd, in1=rstd,
                                            op=mybir.AluOpType.mult)
                    nc.vector.tensor_tensor(out=rt, in0=rt, in1=veps,
                                            op=mybir.AluOpType.mult)
                    nc.vector.tensor_scalar(out=rt, in0=rt, scalar1=-0.5,
                                            scalar2=1.5,
                                            op0=mybir.AluOpType.mult,
                                            op1=mybir.AluOpType.add)
                    nc.vector.tensor_tensor(out=rstd, in0=rstd, in1=rt,
                                            op=mybir.AluOpType.mult)
                return rstd

            def ln_group(grp, l, which, slots):
                """LN for all chunks in grp -> normalized bf16 transposed into
                slots[0:64, g, :] (row 64 = persistent ones). Transposes stage
                through TRS[0:64, 0:256] per pair."""
                n = len(grp)
                mv_g = sb2.tile([128, n, 2], F32, name=f"mv{which}")
                for g, i in enumerate(grp):
                    stats = sb4.tile([128, 6], F32, name=f"st{which}")
                    nc.vector.bn_stats(out=stats, in_=x_sb[:, i, :])
                    nc.vector.bn_aggr(out=mv_g[:, g, :], in_=stats)
                rstd = batched_rstd(mv_g, n, which)
                for gp in range(0, n, 2):
                    n2 = min(2, n - gp)
                    for g2 in range(n2):
                        g = gp + g2
                        xn = sb4.tile([128, C], BF16, name=f"xn{which}")
                        nc.vector.tensor_scalar(
                            out=xn, in0=x_sb[:, grp[g], :],
                            scalar1=mv_g[:, g, 0:1], scalar2=rstd[:, g:g + 1],
                            op0=mybir.AluOpType.subtract,
                            op1=mybir.AluOpType.mult)
                        nc.tensor.transpose(out=TRS[0:64, g2 * 128:(g2 + 1) * 128],
                                            in_=xn, identity=idb_sb)
                    nc.vector.tensor_copy(out=slots[0:64, gp:gp + n2, :],
                                          in_=TRS[0:64, 0:n2 * 128])

            # ---- lm head: incremental unit emission ----
            # each unit = 2x 500-col mms into one side's 2-bank tile + one
            # 1000-col evac (DVE from LMD, ACT from LMA) + stage/DMA.
            lm_pending = []   # (chunk, k) units awaiting emission
            lm_stage = {}     # chunk -> current stage tile

            def queue_lm(i):
                """Prepare xt for chunk i (x_sb[:,i] final) and queue units."""
                xb = sb4.tile([128, C], BF16, name="xb_lm")
                nc.vector.tensor_copy(out=xb, in_=x_sb[:, i, :])
                sl = i % 4
                trr = 512 + (sl % 2) * 128
                nc.tensor.transpose(out=TRS[0:64, trr:trr + 128],
                                    in_=xb, identity=idb_sb)
                nc.vector.tensor_copy(out=xt_sl[0:64, sl, :],
                                      in_=TRS[0:64, trr:trr + 128])
                for k in range(2 * NVC):
                    lm_pending.append((i, k))

            def drain_lm(n):
                """Emit n 500-col lm units. side = k%2 (DVE/ACT), bank =
                (k//2)%2 within that side's tile -> each side ping-pongs its
                own two banks so mm(bank B) overlaps evac(bank A)."""
                for _ in range(min(n, len(lm_pending))):
                    i, k = lm_pending.pop(0)
                    xt = xt_sl[:, i % 4, :]
                    dve = (k % 8) in (0, 3, 6)
                    lg = LMD if dve else LMA
                    nc.tensor.matmul(
                        out=lg[:, 0:500], lhsT=xt,
                        rhs=wlm_sb[:, k * 500:(k + 1) * 500],
                        start=True, stop=True)
                    if k % (SG // 500) == 0:
                        lm_stage[i] = stagep.tile([128, SG], BF16, name="lg_stage")
                    st = lm_stage[i]
                    u = k % (SG // 500)
                    dst = st[:, u * 500:(u + 1) * 500]
                    if dve:
                        nc.vector.tensor_copy(out=dst, in_=lg[:, 0:500])
                    else:
                        nc.scalar.copy(out=dst, in_=lg[:, 0:500])
                    if (k + 1) % (SG // 500) == 0:
                        kg = k // (SG // 500)
                        nc.gpsimd.dma_start(
                            out=out_d[ts(i, 128), kg * SG:(kg + 1) * SG],
                            in_=st)

            qt_of, pt_of = {}, {}

            for gb in range(0, NT, 4):
                grp = [gb, gb + 1, gb + 2, gb + 3]
                for l in range(L):
                    # phase 1: LN1 + q/k/v
                    ln_group(grp, l, 1, xnt_sl)
                    drain_lm(8)
                    for gp in range(0, len(grp), 2):
                        pair = grp[gp:gp + 2]
                        for g2, i in enumerate(pair):
                            xnt = xnt_sl[:, gp + g2, :]
                            tu = pair_use()
                            for h in range(H):
                                nc.tensor.matmul(out=tu[:, 0, ts(h, 128)],
                                                 lhsT=wq_sb[:, l, h, :],
                                                 rhs=xnt, start=True, stop=True)
                            qt = sb4.tile([128, 512], BF16, name="qt")
                            qt_of[i] = qt
                            nc.vector.tensor_copy(out=qt, in_=tu[:, 0, :])
                            kb = (i % 2) * 256
                            nc.tensor.matmul(out=SMALL[:, kb:kb + 128],
                                             lhsT=wk_sb[:, l, :], rhs=xnt,
                                             start=True, stop=True)
                            nc.tensor.matmul(out=SMALL[:, kb + 128:kb + 256],
                                             lhsT=xnt, rhs=wv_sb[:, l, :],
                                             start=True, stop=True)
                            nc.vector.tensor_copy(out=kv_sb[l][:, i, :],
                                                  in_=SMALL[:, kb:kb + 256])

                    # phase 2: scores + exp (paired 1024-col exps)
                    for i in grp:
                        drain_lm(2)
                        qt = qt_of[i]
                        pt = ptp.tile([128, (i + 1) * 512], BF16, name="pt")
                        pt_of[i] = pt
                        nj = i + 1
                        for jp in range(0, nj, 2):
                            tu = pair_use()
                            w = min(2, nj - jp)
                            for s in range(w):
                                j = jp + s
                                if j == i:
                                    nc.tensor.matmul(out=tu[:, s, :], lhsT=idb_sb,
                                                     rhs=mask_sb, start=True,
                                                     stop=False)
                                nc.tensor.matmul(
                                    out=tu[:, s, :],
                                    lhsT=kv_sb[l][:, j, 0:128], rhs=qt,
                                    start=(j != i), stop=True)
                            src_ = tu if w == 2 else tu[:, 0, :]
                            nc.scalar.activation(
                                out=pt[:, jp * 512:(jp + w) * 512],
                                in_=src_,
                                func=mybir.ActivationFunctionType.Exp,
                                scale=SCALE)

                    # phase 3: av (out [t, hd]) + normalize + wo + residual
                    drain_lm(8)
                    for gp in range(0, len(grp), 2):
                        pair = grp[gp:gp + 2]
                        n2 = len(pair)
                        atu = pair_use()
                        at2 = atu[:, 0, 0:256]  # [t, 2, 128]
                        for g2, i in enumerate(pair):
                            pt = pt_of[i]
                            for h in range(H):
                                for j in range(i + 1):
                                    nc.tensor.matmul(
                                        out=atu[:, 0, g2 * 128 + 32 * h:
                                                g2 * 128 + 32 * h + 32],
                                        lhsT=pt[:, (4 * j + h) * 128:(4 * j + h + 1) * 128],
                                        rhs=kv_sb[l][:, j, 128 + 32 * h:160 + 32 * h],
                                        start=(j == 0), stop=(j == i))
                        # reciprocal of the 4 den cols (32h) per chunk
                        rec = sb2.tile([128, n2 * 4], F32, name="rec")
                        den_ap = bass.AP(tensor=at2.tensor, offset=at2.offset,
                                         ap=[at2.ap[0], [128, n2], [32, 4]])
                        nc.vector.reciprocal(out=rec, in_=den_ap)
                        # at_n[t,(g,h,d)] = at2[t,(g,32h+1+d)] * rec[t,(g,h)] bcast d
                        at_n = sb2.tile([128, n2, 64], BF16, name="at_n")
                        in0 = bass.AP(tensor=at2.tensor, offset=at2.offset + 1,
                                      ap=[at2.ap[0], [128, n2], [32, 4], [1, 16]])
                        in1 = bass.AP(tensor=rec.tensor, offset=rec.offset,
                                      ap=[rec.ap[0], [4, n2], [1, 4], [0, 16]])
                        nc.vector.tensor_tensor(out=at_n, in0=in0, in1=in1,
                                                op=mybir.AluOpType.mult)
                        for g2 in range(n2):
                            nc.tensor.transpose(
                                out=TRS[0:64, 256 + g2 * 128:384 + g2 * 128],
                                in_=at_n[:, g2, :], identity=idb_sb)
                        nc.vector.tensor_copy(out=att_sl[:, gp:gp + n2, :],
                                              in_=TRS[0:64, 256:256 + n2 * 128])
                        for g2 in range(n2):
                            nc.tensor.matmul(out=xoy[:, g2 * C:(g2 + 1) * C],
                                             lhsT=att_sl[:, gp + g2, :],
                                             rhs=wo_sb[:, l, :],
                                             start=True, stop=True)
                        xg = bass.AP(tensor=x_sb.tensor,
                                     offset=x_sb.offset + pair[0] * C,
                                     ap=[x_sb.ap[0], [C, n2], [1, C]])
                        nc.vector.tensor_tensor(out=xg, in0=xg,
                                                in1=xoy[:, 0:n2 * C],
                                                op=mybir.AluOpType.add)
                        if apply_bo:
                            for g2, i in enumerate(pair):
                                nc.vector.tensor_add(out=x_sb[:, i, :],
                                                     in0=x_sb[:, i, :],
                                                     in1=bo_sb[:, l, :])

                    # phase 4: LN2 + MLP (+ lm queueing after last layer)
                    ln_group(grp, l, 2, xn2t_sl)
                    drain_lm(8)
                    for gp in range(0, len(grp), 2):
                        pair = grp[gp:gp + 2]
                        n2 = len(pair)
                        tu = pair_use()
                        for g2 in range(n2):
                            for n in range(2):
                                nc.tensor.matmul(
                                    out=tu[:, 0, g2 * 256 + n * 128:
                                           g2 * 256 + (n + 1) * 128],
                                    lhsT=w1_sb[:, l, ts(n, 128)],
                                    rhs=xn2t_sl[:, gp + g2, :],
                                    start=True, stop=True)
                        h_sb = sb4.tile([128, 512], BF16, name="h_sb")
                        nc.vector.tensor_scalar(out=h_sb, in0=tu[:, 0, :],
                                                scalar1=0.0, scalar2=None,
                                                op0=mybir.AluOpType.max)
                        for g2 in range(n2):
                            for n in range(2):
                                nc.tensor.matmul(
                                    out=xoy[:, g2 * C:(g2 + 1) * C],
                                    lhsT=h_sb[:, g2 * 256 + n * 128:
                                              g2 * 256 + (n + 1) * 128],
                                    rhs=w2_sb[:, l, n, :],
                                    start=(n == 0), stop=(n == 1))
                        xg = bass.AP(tensor=x_sb.tensor,
                                     offset=x_sb.offset + pair[0] * C,
                                     ap=[x_sb.ap[0], [C, n2], [1, C]])
                        nc.vector.tensor_tensor(out=xg, in0=xg,
                                                in1=xoy[:, 0:n2 * C],
                                                op=mybir.AluOpType.add)
                        if apply_b2:
                            for g2, i in enumerate(pair):
                                nc.vector.tensor_add(out=x_sb[:, i, :],
                                                     in0=x_sb[:, i, :],
                                                     in1=b2_sb[:, l, :])
                        if l == L - 1:
                            for i in pair:
                                queue_lm(i)
            drain_lm(len(lm_pending))
    nc.compile()
    return nc


def _prep_inputs(idx, tok_emb, pos_emb, Wq, Wk, Wv, Wo, bo, W1, b1, W2, b2,
                 ln1_g, ln1_b, ln2_g, ln2_b, Wlm, blm):
    """Host-side weight layout prep."""
    f32 = np.float32
    bf16 = ml_dtypes.bfloat16
    Wq, Wk, Wv, Wo = f32(Wq), f32(Wk), f32(Wv), f32(Wo)
    W1, W2, Wlm = f32(W1), f32(W2), f32(Wlm)
    ln1_g, ln1_b, ln2_g, ln2_b = f32(ln1_g), f32(ln1_b), f32(ln2_g), f32(ln2_b)
    bo, b1, b2, blm = f32(bo), f32(b1), f32(b2), f32(blm)

    wq_np = np.zeros((L, 65, H, 128), f32)
    wk_np = np.zeros((L, 65, 128), f32)
    wv_np = np.zeros((L, 65, 128), f32)
    wo_np = np.zeros((L, 64, C), f32)
    w1_np = np.zeros((L, 65, FF), f32)
    for l in range(L):
        g1, b1n = ln1_g[l], ln1_b[l]
        g2, b2n = ln2_g[l], ln2_b[l]
        for h in range(H):
            # q: col-block h gets q_h at output rows 32h..32h+16
            wq_np[l, 0:C, h, 32 * h:32 * h + D] = g1[:, None] * Wq[l, h]
            wq_np[l, 64, h, 32 * h:32 * h + D] = b1n @ Wq[l, h]
            # k stack: head h's k lands on rows 32h..32h+16
            wk_np[l, 0:C, 32 * h:32 * h + D] = g1[:, None] * Wk[l, h]
            wk_np[l, 64, 32 * h:32 * h + D] = b1n @ Wk[l, h]
            # v: den ones at col 32h, v at 32h+1..32h+16
            wv_np[l, 0:C, 32 * h + 1:32 * h + 1 + D] = g1[:, None] * Wv[l, h]
            wv_np[l, 64, 32 * h + 1:32 * h + 1 + D] = b1n @ Wv[l, h]
            wv_np[l, 64, 32 * h] = 1.0
            # wo compact: rows 16h+d (matches at_n col order)
            wo_np[l, 16 * h:16 * h + D, :] = Wo[l, 16 * h:16 * h + D, :]
        w1_np[l, 0:C, :] = g2[:, None] * W1[l]
        w1_np[l, 64, :] = b2n @ W1[l] + b1[l]     # ln2 bias fold + b1 fold
    w2_np = W2.reshape(L, 2, 128, C)

    sidx = np.arange(128)
    mask_np = np.where(sidx[:, None] <= sidx[None, :], 0.0, NEG).astype(f32)
    mask_np = np.tile(mask_np, (1, H))
    ident_np = np.eye(128, dtype=f32)

    common = {
        "tok_emb": np.ascontiguousarray(tok_emb, f32),
        "pos_emb": np.ascontiguousarray(pos_emb, f32),
        "wq": np.ascontiguousarray(wq_np.transpose(1, 0, 2, 3)).astype(bf16),
        "wk": np.ascontiguousarray(wk_np.transpose(1, 0, 2)).astype(bf16),
        "wv": np.ascontiguousarray(wv_np.transpose(1, 0, 2)).astype(bf16),
        "wo": np.ascontiguousarray(wo_np.transpose(1, 0, 2)).astype(bf16),
        "w1": np.ascontiguousarray(w1_np.transpose(1, 0, 2)).astype(bf16),
        "w2": np.ascontiguousarray(w2_np.transpose(2, 0, 1, 3)).astype(bf16),
        "maskt": mask_np.astype(bf16),
        "identb": ident_np.astype(bf16),
    }
    apply_bo = bool(np.any(bo != 0))
    apply_b2 = bool(np.any(b2 != 0))
    if apply_bo:
        common["bo_bc"] = np.ascontiguousarray(bo, f32)
    if apply_b2:
        common["b2_bc"] = np.ascontiguousarray(b2, f32)

    wlm_aug = np.concatenate([Wlm, blm[None, :]], axis=0)  # [65, V]
    idx_i = np.asarray(idx).astype(np.int32)

    per_core = []
    for c in range(8):
        b, half = c // 2, c % 2
        m = dict(common)
        m["idx"] = np.ascontiguousarray(idx_i[b].reshape(NT, 128).T)
        m["wlm"] = np.ascontiguousarray(
            wlm_aug[:, half * VH:(half + 1) * VH]).astype(bf16)
        per_core.append(m)
    return per_core, apply_bo, apply_b2


def kernel(**inputs):
    global LAST_RESULTS
    per_core, apply_bo, apply_b2 = _prep_inputs(**inputs)

    key = (apply_bo, apply_b2)
    if key not in _CACHE:
        _CACHE[key] = _build(apply_bo, apply_b2)
    nc = _CACHE[key]

    trace = os.environ.get("KERNEL_TRACE", "0") == "1"
    if trace:
        try:
            from antenv.axon_hooks import get_axon_ntff_profile_hook  # noqa: F401
        except ImportError:
            trace = False
    res = run_bass_kernel_spmd(nc, per_core, core_ids=list(range(8)), trace=trace)
    LAST_RESULTS = res

    out = np.empty((4, T, V), np.float32)
    for c in range(8):
        b, half = c // 2, c % 2
        out[b, :, half * VH:(half + 1) * VH] = np.float32(res.results[c]["logits"])
    return out


# revision 24
# speedup vs baseline: 1.0394x; 1.0394x over previous
"""Bass/Tile kernel for a 4-layer decoder transformer + 32k-vocab LM head on 8 trn2 cores.

Sharding: data-parallel over batch in pairs (core c -> batch c//2), with the
lm_head vocab dim split across each pair (core c -> vocab half c%2). Tiny
transformer params replicated; each core computes the 64-dim transformer for
its sequence and produces [1024, 16000] bf16 logits. Host reassembles
[4, 1024, 32000] f32.

v2 redesign (vs v1 310us): the kernel is PSUM-evacuation bound (every f32
PSUM result needs a DVE/ACT op whose cost = free-size cols + fixed ~0.3us).
  - LM head: one 1000-col bf16 matmul per vocab chunk into per-engine 2-bank
    PSUM tiles (lm_dve/lm_act) so DVE and ACT evacuate concurrently without
    shared-tile read serialization; evac ratio is tunable.
  - Scores: [128,2,512] f32 PSUM tile; ONE Exp per bank-pair (1024 cols)
    halves ACT fixed costs. Causal mask still accumulated via identity mm.
  - av recomputed as out[t, hd] (rhs = v-slice, 32-col outs) -> 4x cheaper on
    PE than [hd, t]; normalization via strided reciprocal + ONE 0-stride
    broadcast tensor_tensor; transpose back for the wo projection.
  - All small evacs batched over chunk pairs/quads (kv fused k+v, batched
    transpose evacuation, batched residual adds, batched relu) to amortize
    per-op fixed costs.
  - b1 folded into w1 row 64 (ones row of xn2t) - relu needs no bias.
  - LN rstd (0x5f3759df + 2 Newton steps) batched over 4 chunks.
"""

import os

import numpy as np
import ml_dtypes

import concourse.bass as bass
import concourse.mybir as mybir
import concourse.tile as tile
from concourse import bacc
from concourse.bass import ts
from concourse.bass_utils import run_bass_kernel_spmd

# model dims
T = 1024
C = 64
H = 4
D = 16
L = 4
FF = 256
V = 32000
VH = V // 2          # per-core vocab half
NT = T // 128        # 8 token chunks
VC = 1000            # lm-head vocab chunk per matmul (one 2-bank psum tile)
NVC = VH // VC       # 16 vocab chunks per core
SG = 4000            # logits staged per output DMA
SCALE = 1.0 / (C ** 0.5)
NEG = -1.0e9

F32 = mybir.dt.float32
BF16 = mybir.dt.bfloat16
I32 = mybir.dt.int32

# of the 16 VC-chunks per token chunk, how many are evacuated by DVE
# (the rest go to ACT) -- balance knob
LM_DVE = 8

_CACHE = {}
LAST_RESULTS = None


def _build(apply_bo, apply_b2):
    nc = bacc.Bacc("TRN2", target_bir_lowering=False, debug=False)

    # ---- DRAM I/O ----
    idx_d = nc.dram_tensor("idx", [128, NT], I32, kind="ExternalInput").ap()
    temb_d = nc.dram_tensor("tok_emb", [V, C], F32, kind="ExternalInput").ap()
    pemb_d = nc.dram_tensor("pos_emb", [T, C], F32, kind="ExternalInput").ap()
    wq_d = nc.dram_tensor("wq", [65, L, H, 128], BF16, kind="ExternalInput").ap()
    wk_d = nc.dram_tensor("wk", [65, L, 128], BF16, kind="ExternalInput").ap()
    wv_d = nc.dram_tensor("wv", [65, L, 128], BF16, kind="ExternalInput").ap()
    wo_d = nc.dram_tensor("wo", [64, L, C], BF16, kind="ExternalInput").ap()
    w1_d = nc.dram_tensor("w1", [65, L, FF], BF16, kind="ExternalInput").ap()
    w2_d = nc.dram_tensor("w2", [128, L, 2, C], BF16, kind="ExternalInput").ap()
    wlm_d = nc.dram_tensor("wlm", [65, VH], BF16, kind="ExternalInput").ap()
    mask_d = nc.dram_tensor("maskt", [128, 512], BF16, kind="ExternalInput").ap()
    idb_d = nc.dram_tensor("identb", [128, 128], BF16, kind="ExternalInput").ap()
    bo_d = b2_d = None
    if apply_bo:
        bo_d = nc.dram_tensor("bo_bc", [L, C], F32, kind="ExternalInput").ap()
    if apply_b2:
        b2_d = nc.dram_tensor("b2_bc", [L, C], F32, kind="ExternalInput").ap()
    out_d = nc.dram_tensor("logits", [T, VH], BF16, kind="ExternalOutput").ap()

    with tile.TileContext(nc) as tc:
        with (
            tc.tile_pool(name="singles", bufs=1) as singles,
            tc.tile_pool(name="sb2", bufs=3) as sb2,
            tc.tile_pool(name="sb4", bufs=4) as sb4,
            tc.tile_pool(name="ptp", bufs=5) as ptp,
            tc.tile_pool(name="stage", bufs=4) as stagep,
            tc.tile_pool(name="pss", bufs=1, space="PSUM") as pss,
        ):
            # ---- persistent PSUM tiles, manually slotted (region-granular
            # deps avoid pool-ring serialization) ----
            # SC1/SC2: alternating 2-bank pair-tiles for q / score-pairs /
            # mlp-h / at2 -- exp of tile A overlaps matmul fill of tile B
            SC1 = pss.tile([128, 2, 512], F32, name="SC1")     # banks 0-1
            SC2 = pss.tile([128, 2, 512], F32, name="SC2")     # banks 2-3
            LMD = pss.tile([128, 512], F32, name="LMD")        # bank 4
            LMA = pss.tile([128, 512], F32, name="LMA")        # bank 5
            # TRS: bf16 transpose staging + xo/y f32 region via bitcast
            TRS = pss.tile([128, 1024], BF16, name="TRS")      # bank 6
            SMALL = pss.tile([128, 512], F32, name="SMALL")    # bank 7
            xoy = TRS[:, 768:1024].bitcast(F32)                # [128,128] f32
            _scu = [0]

            def pair_use():
                t = SC1 if (_scu[0] & 1) == 0 else SC2
                _scu[0] += 1
                return t
            # ---- resident SBUF tensors ----
            wq_sb = singles.tile([65, L, H, 128], BF16, name="wq_sb")
            wk_sb = singles.tile([65, L, 128], BF16, name="wk_sb")
            wv_sb = singles.tile([65, L, 128], BF16, name="wv_sb")
            wo_sb = singles.tile([64, L, C], BF16, name="wo_sb")
            w1_sb = singles.tile([65, L, FF], BF16, name="w1_sb")
            w2_sb = singles.tile([128, L, 2, C], BF16, name="w2_sb")
            wlm_sb = singles.tile([65, VH], BF16, name="wlm_sb")
            mask_sb = singles.tile([128, 512], BF16, name="mask_sb")
            idb_sb = singles.tile([128, 128], BF16, name="idb_sb")
            idx_sb = singles.tile([128, NT], I32, name="idx_sb")
            x_sb = singles.tile([128, NT, C], F32, name="x_sb")
            # fused k|v cache: [0:128] k as [dstack, t]; [128:256] v as [s, dstack]
            kv_sb = [singles.tile([128, NT, 256], BF16, name=f"kv{l}") for l in range(L)]
            # transposed-normalized-x slots with persistent ones row (row 64)
            xnt_sl = singles.tile([65, 4, 128], BF16, name="xnt_sl")
            xn2t_sl = singles.tile([65, 4, 128], BF16, name="xn2t_sl")
            att_sl = singles.tile([64, 4, 128], BF16, name="att_sl")
            xt_sl = singles.tile([65, 4, 128], BF16, name="xt_sl")

            nc.sync.dma_start(out=idx_sb, in_=idx_d)
            pos_sb = singles.tile([128, NT, C], F32, name="pos_sb")
            nc.sync.dma_start(
                out=pos_sb,
                in_=bass.AP(tensor=pemb_d.tensor, offset=pemb_d.offset,
                            ap=[[C, 128], [128 * C, NT], [1, C]]))
            nc.sync.dma_start(out=wq_sb, in_=wq_d)
            nc.sync.dma_start(out=wk_sb, in_=wk_d)
            nc.sync.dma_start(out=wv_sb, in_=wv_d)
            nc.sync.dma_start(out=idb_sb, in_=idb_d)
            nc.sync.dma_start(out=mask_sb, in_=mask_d)
            nc.sync.dma_start(out=wo_sb, in_=wo_d)
            nc.sync.dma_start(out=w1_sb, in_=w1_d)
            nc.sync.dma_start(out=w2_sb, in_=w2_d)
            nc.sync.dma_start(out=wlm_sb, in_=wlm_d)
            nc.vector.memset(xnt_sl[64:65, :, :], 1.0)
            nc.vector.memset(xn2t_sl[64:65, :, :], 1.0)
            nc.vector.memset(xt_sl[64:65, :, :], 1.0)

            bo_sb = b2_sb = None
            if apply_bo:
                bo_sb = singles.tile([128, L, C], F32, name="bo_sb")
                for l in range(L):
                    row = bo_d[l: l + 1, :]
                    nc.gpsimd.dma_start(
                        out=bo_sb[:, l, :],
                        in_=bass.AP(tensor=row.tensor, offset=row.offset,
                                    ap=[[0, 128], [1, C]]))
            if apply_b2:
                b2_sb = singles.tile([128, L, C], F32, name="b2_sb")
                for l in range(L):
                    row = b2_d[l: l + 1, :]
                    nc.gpsimd.dma_start(
                        out=b2_sb[:, l, :],
                        in_=bass.AP(tensor=row.tensor, offset=row.offset,
                                    ap=[[0, 128], [1, C]]))

            # ---- embeddings: x = tok_emb[idx] + pos_emb ----
            for i in range(NT):
                nc.gpsimd.indirect_dma_start(
                    out=x_sb[:, i, :],
                    out_offset=None,
                    in_=temb_d,
                    in_offset=bass.IndirectOffsetOnAxis(ap=idx_sb[:, i:i + 1], axis=0),
                )
                nc.vector.tensor_add(out=x_sb[:, i, :], in0=x_sb[:, i, :],
                                     in1=pos_sb[:, i, :])

            def batched_rstd(mv_g, n, name):
                """rstd[:, 0:n] = (var + eps)^-0.5 for n chunks at once.
                0x5f3759df bit-seed + two Newton steps, all on DVE."""
                veps = sb2.tile([128, n], F32, name=f"veps{name}")
                sh = sb2.tile([128, n], I32, name=f"sh{name}")
                rt = sb2.tile([128, n], F32, name=f"rt{name}")
                rstd = sb2.tile([128, n], F32, name=f"rstd{name}")
                var_ap = bass.AP(tensor=mv_g.tensor, offset=mv_g.offset + 1,
                                 ap=[mv_g.ap[0], [2, n]])
                nc.vector.tensor_scalar(out=veps, in0=var_ap,
                                        scalar1=1e-5, scalar2=None,
                                        op0=mybir.AluOpType.add)
                nc.vector.tensor_scalar(out=sh, in0=veps.bitcast(I32),
                                        scalar1=1, scalar2=None,
                                        op0=mybir.AluOpType.arith_shift_right)
                nc.vector.tensor_scalar(out=sh, in0=sh, scalar1=0, scalar2=None,
                                        op0=mybir.AluOpType.bitwise_not)
                nc.vector.tensor_scalar(out=rstd.bitcast(I32), in0=sh,
                                        scalar1=0x5f3759df + 1, scalar2=None,
                                        op0=mybir.AluOpType.add)
                for _ in range(2):
                    nc.vector.tensor_tensor(out=rt, in0=rstd, in1=rstd,
                                            op=mybir.AluOpType.mult)
                    nc.vector.tensor_tensor(out=rt, in0=rt, in1=veps,
                                            op=mybir.AluOpType.mult)
                    nc.vector.tensor_scalar(out=rt, in0=rt, scalar1=-0.5,
                                            scalar2=1.5,
                                            op0=mybir.AluOpType.mult,
                                            op1=mybir.AluOpType.add)
                    nc.vector.tensor_tensor(out=rstd, in0=rstd, in1=rt,
                                            op=mybir.AluOpType.mult)
                return rstd

            def ln_group(grp, l, which, slots):
                """LN for all chunks in grp -> normalized bf16 transposed into
                slots[0:64, g, :] (row 64 = persistent ones). Transposes stage
                through TRS[0:64, 0:256] per pair."""
                n = len(grp)
                mv_g = sb2.tile([128, n, 2], F32, name=f"mv{which}")
                for g, i in enumerate(grp):
                    stats = sb4.tile([128, 6], F32, name=f"st{which}")
                    nc.vector.bn_stats(out=stats, in_=x_sb[:, i, :])
                    nc.vector.bn_aggr(out=mv_g[:, g, :], in_=stats)
                rstd = batched_rstd(mv_g, n, which)
                for gp in range(0, n, 2):
                    n2 = min(2, n - gp)
                    for g2 in range(n2):
                        g = gp + g2
                        xn = sb4.tile([128, C], BF16, name=f"xn{which}")
                        nc.vector.tensor_scalar(
                            out=xn, in0=x_sb[:, grp[g], :],
                            scalar1=mv_g[:, g, 0:1], scalar2=rstd[:, g:g + 1],
                            op0=mybir.AluOpType.subtract,
                            op1=mybir.AluOpType.mult)
                        nc.tensor.transpose(out=TRS[0:64, g2 * 128:(g2 + 1) * 128],
                                            in_=xn, identity=idb_sb)
                    nc.vector.tensor_copy(out=slots[0:64, gp:gp + n2, :],
                                          in_=TRS[0:64, 0:n2 * 128])

            # ---- lm head: incremental unit emission ----
            # each unit = 2x 500-col mms into one side's 2-bank tile + one
            # 1000-col evac (DVE from LMD, ACT from LMA) + stage/DMA.
            lm_pending = []   # (chunk, k) units awaiting emission
            lm_stage = {}     # chunk -> current stage tile

            def queue_lm(i):
                """Prepare xt for chunk i (x_sb[:,i] final) and queue units."""
                xb = sb4.tile([128, C], BF16, name="xb_lm")
                nc.vector.tensor_copy(out=xb, in_=x_sb[:, i, :])
                sl = i % 4
                trr = 512 + (sl % 2) * 128
                nc.tensor.transpose(out=TRS[0:64, trr:trr + 128],
                                    in_=xb, identity=idb_sb)
                nc.vector.tensor_copy(out=xt_sl[0:64, sl, :],
                                      in_=TRS[0:64, trr:trr + 128])
                for k in range(2 * NVC):
                    lm_pending.append((i, k))

            def drain_lm(n):
                """Emit n 500-col lm units. side = k%2 (DVE/ACT), bank =
                (k//2)%2 within that side's tile -> each side ping-pongs its
                own two banks so mm(bank B) overlaps evac(bank A)."""
                for _ in range(min(n, len(lm_pending))):
                    i, k = lm_pending.pop(0)
                    xt = xt_sl[:, i % 4, :]
                    dve = k % 2 == 0
                    lg = LMD if dve else LMA
                    nc.tensor.matmul(
                        out=lg[:, 0:500], lhsT=xt,
                        rhs=wlm_sb[:, k * 500:(k + 1) * 500],
                        start=True, stop=True)
                    if k % (SG // 500) == 0:
                        lm_stage[i] = stagep.tile([128, SG], BF16, name="lg_stage")
                    st = lm_stage[i]
                    u = k % (SG // 500)
                    dst = st[:, u * 500:(u + 1) * 500]
                    if dve:
                        nc.vector.tensor_copy(out=dst, in_=lg[:, 0:500])
                    else:
                        nc.scalar.copy(out=dst, in_=lg[:, 0:500])
                    if (k + 1) % (SG // 500) == 0:
                        kg = k // (SG // 500)
                        nc.gpsimd.dma_start(
                            out=out_d[ts(i, 128), kg * SG:(kg + 1) * SG],
                            in_=st)

            qt_of, pt_of = {}, {}

            for gb in range(0, NT, 4):
                grp = [gb, gb + 1, gb + 2, gb + 3]
                for l in range(L):
                    # phase 1: LN1 + q/k/v
                    ln_group(grp, l, 1, xnt_sl)
                    drain_lm(8)
                    for gp in range(0, len(grp), 2):
                        pair = grp[gp:gp + 2]
                        for g2, i in enumerate(pair):
                            xnt = xnt_sl[:, gp + g2, :]
                            tu = pair_use()
                            for h in range(H):
                                nc.tensor.matmul(out=tu[:, 0, ts(h, 128)],
                                                 lhsT=wq_sb[:, l, h, :],
                                                 rhs=xnt, start=True, stop=True)
                            qt = sb4.tile([128, 512], BF16, name="qt")
                            qt_of[i] = qt
                            nc.vector.tensor_copy(out=qt, in_=tu[:, 0, :])
                            kb = (i % 2) * 256
                            nc.tensor.matmul(out=SMALL[:, kb:kb + 128],
                                             lhsT=wk_sb[:, l, :], rhs=xnt,
                                             start=True, stop=True)
                            nc.tensor.matmul(out=SMALL[:, kb + 128:kb + 256],
                                             lhsT=xnt, rhs=wv_sb[:, l, :],
                                             start=True, stop=True)
                            nc.vector.tensor_copy(out=kv_sb[l][:, i, :],
                                                  in_=SMALL[:, kb:kb + 256])

                    # phase 2: scores + exp (paired 1024-col exps)
                    for i in grp:
                        qt = qt_of[i]
                        pt = ptp.tile([128, (i + 1) * 512], BF16, name="pt")
                        pt_of[i] = pt
                        nj = i + 1
                        for jp in range(0, nj, 2):
                            tu = pair_use()
                            w = min(2, nj - jp)
                            for s in range(w):
                                j = jp + s
                                if j == i:
                                    nc.tensor.matmul(out=tu[:, s, :], lhsT=idb_sb,
                                                     rhs=mask_sb, start=True,
                                                     stop=False)
                                nc.tensor.matmul(
                                    out=tu[:, s, :],
                                    lhsT=kv_sb[l][:, j, 0:128], rhs=qt,
                                    start=(j != i), stop=True)
                            src_ = tu if w == 2 else tu[:, 0, :]
                            nc.scalar.activation(
                                out=pt[:, jp * 512:(jp + w) * 512],
                                in_=src_,
                                func=mybir.ActivationFunctionType.Exp,
                                scale=SCALE)
                        drain_lm(2)

                    # phase 3: av (out [t, hd]) + normalize + wo + residual
                    drain_lm(8)
                    for gp in range(0, len(grp), 2):
                        pair = grp[gp:gp + 2]
                        n2 = len(pair)
                        atu = pair_use()
                        at2 = atu[:, 0, 0:256]  # [t, 2, 128]
                        for g2, i in enumerate(pair):
                            pt = pt_of[i]
                            for h in range(H):
                                for j in range(i + 1):
                                    nc.tensor.matmul(
                                        out=atu[:, 0, g2 * 128 + 32 * h:
                                                g2 * 128 + 32 * h + 32],
                                        lhsT=pt[:, (4 * j + h) * 128:(4 * j + h + 1) * 128],
                                        rhs=kv_sb[l][:, j, 128 + 32 * h:160 + 32 * h],
                                        start=(j == 0), stop=(j == i))
                        # reciprocal of the 4 den cols (32h) per chunk
                        rec = sb2.tile([128, n2 * 4], F32, name="rec")
                        den_ap = bass.AP(tensor=at2.tensor, offset=at2.offset,
                                         ap=[at2.ap[0], [128, n2], [32, 4]])
                        nc.vector.reciprocal(out=rec, in_=den_ap)
                        # at_n[t,(g,h,d)] = at2[t,(g,32h+1+d)] * rec[t,(g,h)] bcast d
                        at_n = sb2.tile([128, n2, 64], BF16, name="at_n")
                        in0 = bass.AP(tensor=at2.tensor, offset=at2.offset + 1,
                                      ap=[at2.ap[0], [128, n2], [32, 4], [1, 16]])
                        in1 = bass.AP(tensor=rec.tensor, offset=rec.offset,
                                      ap=[rec.ap[0], [4, n2], [1, 4], [0, 16]])
                        nc.vector.tensor_tensor(out=at_n, in0=in0, in1=in1,
                                                op=mybir.AluOpType.mult)
                        for g2 in range(n2):
                            nc.tensor.transpose(
                                out=TRS[0:64, 256 + g2 * 128:384 + g2 * 128],
                                in_=at_n[:, g2, :], identity=idb_sb)
                        nc.vector.tensor_copy(out=att_sl[:, gp:gp + n2, :],
                                              in_=TRS[0:64, 256:256 + n2 * 128])
                        for g2 in range(n2):
                            nc.tensor.matmul(out=xoy[:, g2 * C:(g2 + 1) * C],
                                             lhsT=att_sl[:, gp + g2, :],
                                             rhs=wo_sb[:, l, :],
                                             start=True, stop=True)
                        xg = bass.AP(tensor=x_sb.tensor,
                                     offset=x_sb.offset + pair[0] * C,
                                     ap=[x_sb.ap[0], [C, n2], [1, C]])
                        nc.vector.tensor_tensor(out=xg, in0=xg,
                                                in1=xoy[:, 0:n2 * C],
                                                op=mybir.AluOpType.add)
                        if apply_bo:
                            for g2, i in enumerate(pair):
                                nc.vector.tensor_add(out=x_sb[:, i, :],
                                                     in0=x_sb[:, i, :],
                                                     in1=bo_sb[:, l, :])

                    # phase 4: LN2 + MLP (+ lm queueing after last layer)
                    ln_group(grp, l, 2, xn2t_sl)
                    drain_lm(8)
                    for gp in range(0, len(grp), 2):
                        pair = grp[gp:gp + 2]
                        n2 = len(pair)
                        tu = pair_use()
                        for g2 in range(n2):
                            for n in range(2):
                                nc.tensor.matmul(
                                    out=tu[:, 0, g2 * 256 + n * 128:
                                           g2 * 256 + (n + 1) * 128],
                                    lhsT=w1_sb[:, l, ts(n, 128)],
                                    rhs=xn2t_sl[:, gp + g2, :],
                                    start=True, stop=True)
                        h_sb = sb4.tile([128, 512], BF16, name="h_sb")
                        nc.vector.tensor_scalar(out=h_sb, in0=tu[:, 0, :],
                                                scalar1=0.0, scalar2=None,
                                                op0=mybir.AluOpType.max)
                        for g2 in range(n2):
                            for n in range(2):
                                nc.tensor.matmul(
                                    out=xoy[:, g2 * C:(g2 + 1) * C],
                                    lhsT=h_sb[:, g2 * 256 + n * 128:
                                              g2 * 256 + (n + 1) * 128],
                                    rhs=w2_sb[:, l, n, :],
                                    start=(n == 0), stop=(n == 1))
                        xg = bass.AP(tensor=x_sb.tensor,
                                     offset=x_sb.offset + pair[0] * C,
                                     ap=[x_sb.ap[0], [C, n2], [1, C]])
                        nc.vector.tensor_tensor(out=xg, in0=xg,
                                                in1=xoy[:, 0:n2 * C],
                                                op=mybir.AluOpType.add)
                        if apply_b2:
                            for g2, i in enumerate(pair):
                                nc.vector.tensor_add(out=x_sb[:, i, :],
                                                     in0=x_sb[:, i, :],
                                                     in1=b2_sb[:, l, :])
                        if l == L - 1:
                            for i in pair:
                                queue_lm(i)
            drain_lm(len(lm_pending))
    nc.compile()
    return nc


def _prep_inputs(idx, tok_emb, pos_emb, Wq, Wk, Wv, Wo, bo, W1, b1, W2, b2,
                 ln1_g, ln1_b, ln2_g, ln2_b, Wlm, blm):
    """Host-side weight layout prep."""
    f32 = np.float32
    bf16 = ml_dtypes.bfloat16
    Wq, Wk, Wv, Wo = f32(Wq), f32(Wk), f32(Wv), f32(Wo)
    W1, W2, Wlm = f32(W1), f32(W2), f32(Wlm)
    ln1_g, ln1_b, ln2_g, ln2_b = f32(ln1_g), f32(ln1_b), f32(ln2_g), f32(ln2_b)
    bo, b1, b2, blm = f32(bo), f32(b1), f32(b2), f32(blm)

    wq_np = np.zeros((L, 65, H, 128), f32)
    wk_np = np.zeros((L, 65, 128), f32)
    wv_np = np.zeros((L, 65, 128), f32)
    wo_np = np.zeros((L, 64, C), f32)
    w1_np = np.zeros((L, 65, FF), f32)
    for l in range(L):
        g1, b1n = ln1_g[l], ln1_b[l]
        g2, b2n = ln2_g[l], ln2_b[l]
        for h in range(H):
            # q: col-block h gets q_h at output rows 32h..32h+16
            wq_np[l, 0:C, h, 32 * h:32 * h + D] = g1[:, None] * Wq[l, h]
            wq_np[l, 64, h, 32 * h:32 * h + D] = b1n @ Wq[l, h]
            # k stack: head h's k lands on rows 32h..32h+16
            wk_np[l, 0:C, 32 * h:32 * h + D] = g1[:, None] * Wk[l, h]
            wk_np[l, 64, 32 * h:32 * h + D] = b1n @ Wk[l, h]
            # v: den ones at col 32h, v at 32h+1..32h+16
            wv_np[l, 0:C, 32 * h + 1:32 * h + 1 + D] = g1[:, None] * Wv[l, h]
            wv_np[l, 64, 32 * h + 1:32 * h + 1 + D] = b1n @ Wv[l, h]
            wv_np[l, 64, 32 * h] = 1.0
            # wo compact: rows 16h+d (matches at_n col order)
            wo_np[l, 16 * h:16 * h + D, :] = Wo[l, 16 * h:16 * h + D, :]
        w1_np[l, 0:C, :] = g2[:, None] * W1[l]
        w1_np[l, 64, :] = b2n @ W1[l] + b1[l]     # ln2 bias fold + b1 fold
    w2_np = W2.reshape(L, 2, 128, C)

    sidx = np.arange(128)
    mask_np = np.where(sidx[:, None] <= sidx[None, :], 0.0, NEG).astype(f32)
    mask_np = np.tile(mask_np, (1, H))
    ident_np = np.eye(128, dtype=f32)

    common = {
        "tok_emb": np.ascontiguousarray(tok_emb, f32),
        "pos_emb": np.ascontiguousarray(pos_emb, f32),
        "wq": np.ascontiguousarray(wq_np.transpose(1, 0, 2, 3)).astype(bf16),
        "wk": np.ascontiguousarray(wk_np.transpose(1, 0, 2)).astype(bf16),
        "wv": np.ascontiguousarray(wv_np.transpose(1, 0, 2)).astype(bf16),
        "wo": np.ascontiguousarray(wo_np.transpose(1, 0, 2)).astype(bf16),
        "w1": np.ascontiguousarray(w1_np.transpose(1, 0, 2)).astype(bf16),
        "w2": np.ascontiguousarray(w2_np.transpose(2, 0, 1, 3)).astype(bf16),
        "maskt": mask_np.astype(bf16),
        "identb": ident_np.astype(bf16),
    }
    apply_bo = bool(np.any(bo != 0))
    apply_b2 = bool(np.any(b2 != 0))
    if apply_bo:
        common["bo_bc"] = np.ascontiguousarray(bo, f32)
    if apply_b2:
        common["b2_bc"] = np.ascontiguousarray(b2, f32)

    wlm_aug = np.concatenate([Wlm, blm[None, :]], axis=0)  # [65, V]
    idx_i = np.asarray(idx).astype(np.int32)

    per_core = []
    for c in range(8):
        b, half = c // 2, c % 2
        m = dict(common)
        m["idx"] = np.ascontiguousarray(idx_i[b].reshape(NT, 128).T)
        m["wlm"] = np.ascontiguousarray(
            wlm_aug[:, half * VH:(half + 1) * VH]).astype(bf16)
        per_core.append(m)
    return per_core, apply_bo, apply_b2


def kernel(**inputs):
    global LAST_RESULTS
    per_core, apply_bo, apply_b2 = _prep_inputs(**inputs)

    key = (apply_bo, apply_b2)
    if key not in _CACHE:
        _CACHE[key] = _build(apply_bo, apply_b2)
    nc = _CACHE[key]

    trace = os.environ.get("KERNEL_TRACE", "0") == "1"
    if trace:
        try:
            from antenv.axon_hooks import get_axon_ntff_profile_hook  # noqa: F401
        except ImportError:
            trace = False
    res = run_bass_kernel_spmd(nc, per_core, core_ids=list(range(8)), trace=trace)
    LAST_RESULTS = res

    out = np.empty((4, T, V), np.float32)
    for c in range(8):
        b, half = c // 2, c % 2
        out[b, :, half * VH:(half + 1) * VH] = np.float32(res.results[c]["logits"])
    return out
